# revision 7
# baseline (speedup 1.0000x reference)
"""Trainium2 Bass kernel for ConstructAdjMatrix (GNN message passing).

Math (reference):
    d_x = (rowsum(adj) + 1) ** -0.5          # [N_CELL]
    d_y = (colsum(adj) + 1) ** -0.5          # [N_DRUG]
    agg_cell_lp = d_x[:,None] * adj * d_y    # [N_CELL, N_DRUG]
    agg_drug_lp = agg_cell_lp.T              # [N_DRUG, N_CELL]
    self_cell_lp = diag(1/(rowsum+1) + 1)    # [N_CELL, N_CELL]
    self_drug_lp = diag(1/(colsum+1) + 1)    # [N_DRUG, N_DRUG]

Sharding: adj row-sharded across 8 cores (1024 rows each). Row degrees are
local; column degrees need one 16KB AllReduce across the 8 cores. Each core
writes its row block of agg_cell_lp and (via PE transpose) its column block
of agg_drug_lp. The diag outputs only need their diagonal vectors from the
device; the host assembles the (mostly zero) diag matrices.

Dataflow per core (structured so only d_y-dependent work waits on the
AllReduce):
  A. Load 8 row tiles [128,4096]. DVE accumulates acc += tile with
     accum_out giving cumulative rowsums (per-tile rowsum by difference);
     ACT pre-scales each tile by d_x in place.
  B. PE reduces acc over partitions (8 matmuls with ones) -> colsum;
     copy to SBUF row, DMA to DRAM, AllReduce.
  C. After AllReduce: d_y in f-major layout for per-partition scaling;
     d_y broadcast tile [128,4096] via a partition-stride-0 DMA.
  D. out1 tiles = (d_x-scaled adj) * dyb on DVE -> DMA out.
     PE transposes d_x-scaled tiles chunk-wise into PSUM; ACT applies d_y
     as a per-partition scale on the PSUM->SBUF copy; DMA stage -> out2.
"""

import numpy as np

from concourse import bacc, bass, masks, mybir, tile
from concourse.bass_utils import run_bass_kernel_spmd

N_CELL, N_DRUG = 8192, 4096
NC = 8                 # cores
R = N_CELL // NC       # 1024 rows per core
P = 128                # partitions
NT = R // P            # 8 row tiles of [128, 4096] per core
NB = N_DRUG // 512     # 8 psum banks for the column-sum
ND = N_DRUG // P       # 32 drug chunks of 128 for the transpose
NF = N_DRUG // P       # 32 f-major columns
FP32 = mybir.dt.float32
ADD = mybir.AluOpType.add
MULT = mybir.AluOpType.mult
SUB = mybir.AluOpType.subtract
AXF = mybir.AxisListType.X


def _build_kernel():
    nc = bacc.Bacc(
        "TRN2", target_bir_lowering=False, debug=False, num_devices=NC
    )
    adj = nc.dram_tensor("adj_block", [R, N_DRUG], FP32, kind="ExternalInput").ap()
    out1 = nc.dram_tensor("out1", [R, N_DRUG], FP32, kind="ExternalOutput").ap()
    out2 = nc.dram_tensor("out2", [N_DRUG, R], FP32, kind="ExternalOutput").ap()
    dcell = nc.dram_tensor("dcell", [R], FP32, kind="ExternalOutput").ap()
    ddrug = nc.dram_tensor("ddrug", [N_DRUG], FP32, kind="ExternalOutput").ap()

    with tile.TileContext(nc) as tc:
        _body(tc, adj, out1, out2, dcell, ddrug)
    nc.compile()
    return nc


def _body(tc, adj, out1, out2, dcell, ddrug):
    nc = tc.nc
    from contextlib import ExitStack

    with ExitStack() as ctx:
        const = ctx.enter_context(tc.tile_pool(name="const", bufs=1))
        adj_pool = ctx.enter_context(tc.tile_pool(name="adjp", bufs=1))
        sb = ctx.enter_context(tc.tile_pool(name="sb", bufs=1))
        dram = ctx.enter_context(tc.tile_pool(name="dram", bufs=1, space="DRAM"))

        identity = const.tile([P, P], FP32, name="identity")
        masks.make_identity(nc, identity[:])
        ones_col = const.tile([P, 1], FP32, name="ones_col")
        nc.gpsimd.memset(ones_col[:], 1.0)

        cs_in = dram.tile([N_DRUG], FP32, name="cs_in")
        cs_out = dram.tile([N_DRUG], FP32, name="cs_out", addr_space="Shared")
        dy_dram = dram.tile([N_DRUG], FP32, name="dy_dram")

        cs_ctx = ExitStack()
        acc_pool = cs_ctx.enter_context(tc.tile_pool(name="accp", bufs=1))
        acc = acc_pool.tile([P, N_DRUG], FP32, name="acc")  # running tile sum
        cum = sb.tile([P, NT], FP32, name="cum", tag="cum")    # cumulative rowsums
        r_t = sb.tile([P, NT], FP32, name="r_t", tag="r_t")    # per-tile rowsums
        rrec = sb.tile([P, NT], FP32, name="rrec", tag="rrec")  # 1/(r+1)
        dx = sb.tile([P, NT], FP32, name="dx", tag="dx")
        dc1 = sb.tile([P, NT], FP32, name="dc1", tag="dc1")

        # ---- Phase A: load, accumulate (colsum partial), rowsums, dx scale
        adj_tiles = []
        for t in range(NT):
            at = adj_pool.tile([P, N_DRUG], FP32, name=f"adj{t}", tag=f"adj{t}")
            nc.sync.dma_start(out=at[:], in_=adj[t * P : (t + 1) * P, :])
            adj_tiles.append(at)
            if t == 0:
                nc.vector.scalar_tensor_tensor(
                    out=acc[:], in0=at[:], scalar=0.0, in1=at[:],
                    op0=MULT, op1=ADD, accum_out=cum[:, 0:1],
                )
            else:
                nc.vector.scalar_tensor_tensor(
                    out=acc[:], in0=at[:], scalar=1.0, in1=acc[:],
                    op0=MULT, op1=ADD, accum_out=cum[:, t : t + 1],
                )
            # per-tile rowsum by difference of cumulative sums
            if t == 0:
                nc.vector.tensor_copy(r_t[:, 0:1], cum[:, 0:1])
            else:
                nc.vector.tensor_tensor(
                    out=r_t[:, t : t + 1], in0=cum[:, t : t + 1],
                    in1=cum[:, t - 1 : t], op=SUB,
                )
            nc.vector.tensor_scalar_add(rrec[:, t : t + 1], r_t[:, t : t + 1], 1.0)
            nc.vector.reciprocal(rrec[:, t : t + 1], rrec[:, t : t + 1])
            nc.scalar.sqrt(dx[:, t : t + 1], rrec[:, t : t + 1])
            # pre-scale tile by d_x in place (ACT, per-partition scale)
            nc.scalar.mul(at[:], at[:], dx[:, t : t + 1])

        # self_cell diag values
        nc.vector.tensor_scalar_add(dc1[:], rrec[:], 1.0)
        nc.sync.dma_start(out=dcell.rearrange("(t p) -> p t", p=P), in_=dc1[:])

        # ---- Phase B: colsum = partition-reduce(acc) -> AllReduce ---------
        psum_cs = cs_ctx.enter_context(
            tc.tile_pool(name="psum_cs", bufs=1, space="PSUM")
        )
        cs_row_pool = cs_ctx.enter_context(tc.tile_pool(name="csrp", bufs=1))
        cs_row = cs_row_pool.tile([1, N_DRUG], FP32, name="cs_row")
        for b in range(NB):
            csb = psum_cs.tile([1, 512], FP32, name=f"csb{b}", tag=f"csb{b}")
            nc.tensor.matmul(
                csb[:1, :], ones_col[:], acc[:, b * 512 : (b + 1) * 512]
            )
            nc.any.tensor_copy(cs_row[:1, b * 512 : (b + 1) * 512], csb[:1, :])
        nc.sync.dma_start(out=cs_in[:], in_=cs_row[:1, :])
        cs_ctx.close()
        nc.gpsimd.collective_compute(
            "AllReduce",
            mybir.AluOpType.add,
            replica_groups=[list(range(NC))],
            ins=[cs_in.opt()],
            outs=[cs_out.opt()],
        )

        # ---- Phase C: d_y vectors ----------------------------------------
        # f-major layout: s_ft[p, f] = S[f*128 + p]; chunk d's per-partition
        # scale is dy_ft[:, d].
        s_ft = sb.tile([P, NF], FP32, name="s_ft", tag="s_ft")
        nc.sync.dma_start(out=s_ft[:], in_=cs_out.rearrange("(f p) -> p f", p=P))
        srec = sb.tile([P, NF], FP32, name="srec", tag="srec")
        nc.vector.tensor_scalar_add(srec[:], s_ft[:], 1.0)
        nc.vector.reciprocal(srec[:], srec[:])
        dy_ft = sb.tile([P, NF], FP32, name="dy_ft", tag="dy_ft")
        nc.scalar.sqrt(dy_ft[:], srec[:])
        dd1 = sb.tile([P, NF], FP32, name="dd1", tag="dd1")
        nc.vector.tensor_scalar_add(dd1[:], srec[:], 1.0)
        nc.sync.dma_start(out=ddrug.rearrange("(f p) -> p f", p=P), in_=dd1[:])

        # d_y broadcast [128, N_DRUG] via DRAM roundtrip + stride-0 read
        nc.sync.dma_start(out=dy_dram.rearrange("(f p) -> p f", p=P), in_=dy_ft[:])
        out_ctx = ExitStack()
        late = out_ctx.enter_context(tc.tile_pool(name="late", bufs=1))
        dyb = late.tile([P, N_DRUG], FP32, name="dyb")
        nc.sync.dma_start(out=dyb[:], in_=dy_dram[:].partition_broadcast(P))

        # ---- Phase D: outputs --------------------------------------------
        o1_pool = out_ctx.enter_context(tc.tile_pool(name="o1p", bufs=2))
        psum_tp = out_ctx.enter_context(
            tc.tile_pool(name="psum_tp", bufs=6, space="PSUM")
        )
        stage_pool = out_ctx.enter_context(tc.tile_pool(name="stage", bufs=3))

        # out1 = atx * dyb (DVE) -> DMA
        for t in range(NT):
            o1 = o1_pool.tile([P, N_DRUG], FP32, name=f"o1_{t}", tag="o1")
            nc.vector.tensor_tensor(
                out=o1[:], in0=adj_tiles[t][:], in1=dyb[:], op=MULT
            )
            nc.sync.dma_start(out=out1[t * P : (t + 1) * P, :], in_=o1[:])

        # out2 = transpose(atx) scaled by d_y on the PSUM->SBUF copy (ACT)
        for d in range(ND):
            stg = stage_pool.tile([P, R], FP32, name=f"stg{d}", tag="stg")
            for g in range(2):
                pt = psum_tp.tile([P, 512], FP32, name=f"pt{d}_{g}", tag="ptp")
                for t4 in range(4):
                    t = g * 4 + t4
                    nc.tensor.matmul(
                        pt[:, t4 * P : (t4 + 1) * P],
                        adj_tiles[t][:, d * P : (d + 1) * P],
                        identity[:],
                        is_transpose=True,
                    )
                nc.scalar.mul(
                    stg[:, g * 512 : (g + 1) * 512], pt[:], dy_ft[:, d : d + 1]
                )
            nc.sync.dma_start(out=out2[d * P : (d + 1) * P, :], in_=stg[:])
        out_ctx.close()


_CACHE = {}


def _get_kernel():
    if "nc" not in _CACHE:
        _CACHE["nc"] = _build_kernel()
    return _CACHE["nc"]


def kernel(adj):
    adj = np.ascontiguousarray(np.asarray(adj, dtype=np.float32))
    assert adj.shape == (N_CELL, N_DRUG)
    nc = _get_kernel()
    in_maps = [{"adj_block": adj[c * R : (c + 1) * R]} for c in range(NC)]
    res = run_bass_kernel_spmd(nc, in_maps, list(range(NC))).results

    agg_cell = np.concatenate([res[c]["out1"] for c in range(NC)], axis=0)
    agg_drug = np.concatenate([res[c]["out2"] for c in range(NC)], axis=1)
    self_cell = np.zeros((N_CELL, N_CELL), np.float32)
    np.fill_diagonal(self_cell, np.concatenate([res[c]["dcell"] for c in range(NC)]))
    self_drug = np.zeros((N_DRUG, N_DRUG), np.float32)
    np.fill_diagonal(self_drug, res[0]["ddrug"])
    return (agg_cell, agg_drug, self_cell, self_drug)


# revision 12
# speedup vs baseline: 1.0949x; 1.0949x over previous
"""Trainium2 Bass kernel for ConstructAdjMatrix (GNN message passing).

Math (reference):
    d_x = (rowsum(adj) + 1) ** -0.5          # [N_CELL]
    d_y = (colsum(adj) + 1) ** -0.5          # [N_DRUG]
    agg_cell_lp = d_x[:,None] * adj * d_y    # [N_CELL, N_DRUG]
    agg_drug_lp = agg_cell_lp.T              # [N_DRUG, N_CELL]
    self_cell_lp = diag(1/(rowsum+1) + 1)    # [N_CELL, N_CELL]
    self_drug_lp = diag(1/(colsum+1) + 1)    # [N_DRUG, N_DRUG]

Sharding: adj row-sharded across 8 cores (1024 rows each). Row degrees are
local; column degrees need one 16KB AllReduce across the 8 cores. Each core
writes its row block of agg_cell_lp and (via PE transpose) its column block
of agg_drug_lp. The diag outputs only need their diagonal vectors from the
device; the host assembles the (mostly zero) diag matrices.

Dataflow per core (structured so only d_y-dependent work waits on the
AllReduce):
  A. Load 8 row tiles [128,4096]. DVE accumulates acc += tile with
     accum_out giving cumulative rowsums (per-tile rowsum by difference);
     ACT pre-scales each tile by d_x in place.
  B. PE reduces acc over partitions (8 matmuls with ones) -> colsum;
     copy to SBUF row, DMA to DRAM, AllReduce.
  C. After AllReduce: d_y in f-major layout for per-partition scaling;
     d_y broadcast tile [128,4096] via a partition-stride-0 DMA.
  D. out1 tiles = (d_x-scaled adj) * dyb on DVE -> DMA out.
     PE transposes d_x-scaled tiles chunk-wise into PSUM; ACT applies d_y
     as a per-partition scale on the PSUM->SBUF copy; DMA stage -> out2.
"""

import numpy as np

from concourse import bacc, bass, masks, mybir, tile
from concourse.bass_utils import run_bass_kernel_spmd

N_CELL, N_DRUG = 8192, 4096
NC = 8                 # cores
R = N_CELL // NC       # 1024 rows per core
P = 128                # partitions
NT = R // P            # 8 row tiles of [128, 4096] per core
NB = N_DRUG // 512     # 8 psum banks for the column-sum
ND = N_DRUG // P       # 32 drug chunks of 128 for the transpose
NF = N_DRUG // P       # 32 f-major columns
FP32 = mybir.dt.float32
ADD = mybir.AluOpType.add
MULT = mybir.AluOpType.mult
SUB = mybir.AluOpType.subtract
AXF = mybir.AxisListType.X


def _build_kernel():
    nc = bacc.Bacc(
        "TRN2", target_bir_lowering=False, debug=False, num_devices=NC
    )
    adj = nc.dram_tensor("adj_block", [R, N_DRUG], FP32, kind="ExternalInput").ap()
    out1 = nc.dram_tensor("out1", [R, N_DRUG], FP32, kind="ExternalOutput").ap()
    out2 = nc.dram_tensor("out2", [N_DRUG, R], FP32, kind="ExternalOutput").ap()
    dcell = nc.dram_tensor("dcell", [R], FP32, kind="ExternalOutput").ap()
    ddrug = nc.dram_tensor("ddrug", [N_DRUG], FP32, kind="ExternalOutput").ap()

    with tile.TileContext(nc) as tc:
        _body(tc, adj, out1, out2, dcell, ddrug)
    nc.compile()
    return nc


def _body(tc, adj, out1, out2, dcell, ddrug):
    nc = tc.nc
    from contextlib import ExitStack

    with ExitStack() as ctx:
        const = ctx.enter_context(tc.tile_pool(name="const", bufs=1))
        adj_pool = ctx.enter_context(tc.tile_pool(name="adjp", bufs=1))
        sb = ctx.enter_context(tc.tile_pool(name="sb", bufs=1))
        dram = ctx.enter_context(tc.tile_pool(name="dram", bufs=1, space="DRAM"))

        # identity as an inline constant (gpsimd make_identity costs ~70us
        # and blocks the collective, which also issues from gpsimd)
        ident_dram = nc.inline_tensor(np.eye(P, dtype=np.float32), name="ident")
        identity = const.tile([P, P], FP32, name="identity")
        nc.sync.dma_start(out=identity[:], in_=ident_dram.ap())
        ones_col = const.tile([P, 1], FP32, name="ones_col")
        nc.vector.memset(ones_col[:], 1.0)
        ones_row = const.tile([1, P], FP32, name="ones_row")
        nc.vector.memset(ones_row[:], 1.0)

        cs_in = dram.tile([N_DRUG], FP32, name="cs_in")
        cs_out = dram.tile([N_DRUG], FP32, name="cs_out", addr_space="Shared")

        cs_ctx = ExitStack()
        acc_pool = cs_ctx.enter_context(tc.tile_pool(name="accp", bufs=1))
        acc = acc_pool.tile([P, N_DRUG], FP32, name="acc")  # running tile sum
        cum = sb.tile([P, NT], FP32, name="cum", tag="cum")    # cumulative rowsums
        rrec = sb.tile([P, NT], FP32, name="rrec", tag="rrec")  # 1/(r+1)
        dx = sb.tile([P, NT], FP32, name="dx", tag="dx")
        dc1 = sb.tile([P, NT], FP32, name="dc1", tag="dc1")

        # ---- Phase A: load, accumulate (colsum partial), rowsums, dx scale
        adj_tiles = []
        for t in range(NT):
            at = adj_pool.tile([P, N_DRUG], FP32, name=f"adj{t}", tag=f"adj{t}")
            nc.sync.dma_start(out=at[:], in_=adj[t * P : (t + 1) * P, :])
            adj_tiles.append(at)
            if t == 0:
                nc.vector.scalar_tensor_tensor(
                    out=acc[:], in0=at[:], scalar=0.0, in1=at[:],
                    op0=MULT, op1=ADD, accum_out=cum[:, 0:1],
                )
            else:
                nc.vector.scalar_tensor_tensor(
                    out=acc[:], in0=at[:], scalar=1.0, in1=acc[:],
                    op0=MULT, op1=ADD, accum_out=cum[:, t : t + 1],
                )
            # per-tile rowsum by difference of cumulative sums
            if t == 0:
                nc.vector.tensor_scalar_add(rrec[:, 0:1], cum[:, 0:1], 1.0)
            else:
                nc.vector.tensor_tensor(
                    out=rrec[:, t : t + 1], in0=cum[:, t : t + 1],
                    in1=cum[:, t - 1 : t], op=SUB,
                )
                nc.vector.tensor_scalar_add(
                    rrec[:, t : t + 1], rrec[:, t : t + 1], 1.0
                )
            nc.vector.reciprocal(rrec[:, t : t + 1], rrec[:, t : t + 1])
            nc.scalar.sqrt(dx[:, t : t + 1], rrec[:, t : t + 1])
            # pre-scale tile by d_x in place (ACT, per-partition scale)
            nc.scalar.mul(at[:], at[:], dx[:, t : t + 1])

        # self_cell diag values
        nc.vector.tensor_scalar_add(dc1[:], rrec[:], 1.0)
        nc.sync.dma_start(out=dcell.rearrange("(t p) -> p t", p=P), in_=dc1[:])

        # ---- Phase B: colsum = partition-reduce(acc) -> AllReduce ---------
        psum_cs = cs_ctx.enter_context(
            tc.tile_pool(name="psum_cs", bufs=1, space="PSUM")
        )
        cs_row_pool = cs_ctx.enter_context(tc.tile_pool(name="csrp", bufs=1))
        cs_row = cs_row_pool.tile([1, N_DRUG], FP32, name="cs_row")
        for b in range(NB):
            csb = psum_cs.tile([1, 512], FP32, name=f"csb{b}", tag=f"csb{b}")
            nc.tensor.matmul(
                csb[:1, :], ones_col[:], acc[:, b * 512 : (b + 1) * 512]
            )
            nc.any.tensor_copy(cs_row[:1, b * 512 : (b + 1) * 512], csb[:1, :])
        nc.sync.dma_start(out=cs_in[:], in_=cs_row[:1, :])
        cs_ctx.close()
        nc.gpsimd.collective_compute(
            "AllReduce",
            mybir.AluOpType.add,
            replica_groups=[list(range(NC))],
            ins=[cs_in.opt()],
            outs=[cs_out.opt()],
        )

        # ---- Phase C: d_y vectors ----------------------------------------
        # f-major layout: s_ft[p, f] = S[f*128 + p]; chunk d's per-partition
        # scale is dy_ft[:, d].
        s_ft = sb.tile([P, NF], FP32, name="s_ft", tag="s_ft")
        nc.sync.dma_start(out=s_ft[:], in_=cs_out.rearrange("(f p) -> p f", p=P))
        srec = sb.tile([P, NF], FP32, name="srec", tag="srec")
        nc.vector.tensor_scalar_add(srec[:], s_ft[:], 1.0)
        nc.vector.reciprocal(srec[:], srec[:])
        dy_ft = sb.tile([P, NF], FP32, name="dy_ft", tag="dy_ft")
        nc.scalar.sqrt(dy_ft[:], srec[:])
        dd1 = sb.tile([P, NF], FP32, name="dd1", tag="dd1")
        nc.vector.tensor_scalar_add(dd1[:], srec[:], 1.0)
        nc.sync.dma_start(out=ddrug.rearrange("(f p) -> p f", p=P), in_=dd1[:])

        # d_y broadcast [128, N_DRUG] on-chip: PE-transpose dy_ft to get d_y
        # rows on 32 partitions, then K=1 matmuls with a ones column replicate
        # each row across all 128 partitions (avoids the 26us descriptor-gen
        # cost of a partition-stride-0 DMA).
        out_ctx = ExitStack()
        late = out_ctx.enter_context(tc.tile_pool(name="late", bufs=1))
        psum_misc = out_ctx.enter_context(
            tc.tile_pool(name="psum_misc", bufs=1, space="PSUM")
        )
        dyt_ps = psum_misc.tile([NF, P], FP32, name="dyt_ps", tag="dyt")
        nc.tensor.matmul(dyt_ps[:], dy_ft[:], identity[:], is_transpose=True)
        dy32 = sb.tile([NF, P], FP32, name="dy32", tag="dy32")
        nc.scalar.copy(dy32[:], dyt_ps[:])
        # gather the 32 partition rows into one [1, 4096] row (32 contiguous
        # 512B descriptors; cheap) so K=1 matmul rhs sits at base partition 0
        dy_row = sb.tile([1, N_DRUG], FP32, name="dy_row", tag="dy_row")
        nc.sync.dma_start(out=dy_row[:1, :], in_=dy32[:])
        dyb = late.tile([P, N_DRUG], FP32, name="dyb")
        for b in range(NB):
            pb = psum_misc.tile([P, 512], FP32, name=f"dybp{b}", tag="dybp", bufs=2)
            nc.tensor.matmul(
                pb[:], ones_row[:1, :], dy_row[:1, b * 512 : (b + 1) * 512]
            )
            nc.scalar.copy(dyb[:, b * 512 : (b + 1) * 512], pb[:])

        # ---- Phase D: outputs --------------------------------------------
        o1_pool = out_ctx.enter_context(tc.tile_pool(name="o1p", bufs=2))
        psum_tp = out_ctx.enter_context(
            tc.tile_pool(name="psum_tp", bufs=5, space="PSUM")
        )
        stage_pool = out_ctx.enter_context(tc.tile_pool(name="stage", bufs=3))

        # out1 = atx * dyb (DVE) -> DMA on gpsimd queues (keeps the sync
        # queues free for the stage stores)
        for t in range(NT):
            o1 = o1_pool.tile([P, N_DRUG], FP32, name=f"o1_{t}", tag="o1")
            nc.vector.tensor_tensor(
                out=o1[:], in0=adj_tiles[t][:], in1=dyb[:], op=MULT
            )
            nc.gpsimd.dma_start(out=out1[t * P : (t + 1) * P, :], in_=o1[:])

        # out2 = transpose(atx) scaled by d_y on the PSUM->SBUF copy (ACT)
        for d in range(ND):
            stg = stage_pool.tile([P, R], FP32, name=f"stg{d}", tag="stg")
            for g in range(2):
                pt = psum_tp.tile([P, 512], FP32, name=f"pt{d}_{g}", tag="ptp")
                for t4 in range(4):
                    t = g * 4 + t4
                    nc.tensor.matmul(
                        pt[:, t4 * P : (t4 + 1) * P],
                        adj_tiles[t][:, d * P : (d + 1) * P],
                        identity[:],
                        is_transpose=True,
                    )
                nc.scalar.mul(
                    stg[:, g * 512 : (g + 1) * 512], pt[:], dy_ft[:, d : d + 1]
                )
            nc.sync.dma_start(out=out2[d * P : (d + 1) * P, :], in_=stg[:])
        out_ctx.close()


_CACHE = {}


def _get_kernel():
    if "nc" not in _CACHE:
        _CACHE["nc"] = _build_kernel()
    return _CACHE["nc"]


def kernel(adj):
    adj = np.ascontiguousarray(np.asarray(adj, dtype=np.float32))
    assert adj.shape == (N_CELL, N_DRUG)
    nc = _get_kernel()
    in_maps = [{"adj_block": adj[c * R : (c + 1) * R]} for c in range(NC)]
    res = run_bass_kernel_spmd(nc, in_maps, list(range(NC))).results

    agg_cell = np.concatenate([res[c]["out1"] for c in range(NC)], axis=0)
    agg_drug = np.concatenate([res[c]["out2"] for c in range(NC)], axis=1)
    self_cell = np.zeros((N_CELL, N_CELL), np.float32)
    np.fill_diagonal(self_cell, np.concatenate([res[c]["dcell"] for c in range(NC)]))
    self_drug = np.zeros((N_DRUG, N_DRUG), np.float32)
    np.fill_diagonal(self_drug, res[0]["ddrug"])
    return (agg_cell, agg_drug, self_cell, self_drug)
